# revision 1
# baseline (speedup 1.0000x reference)
"""Trainium2 Bass kernel for the BasicQuadRGBV2 demosaic model.

Data-parallel over batch: 1 image per NeuronCore (8 cores).

Per-core dataflow (image [4,512,512] -> [3,1024,1024]):
  Phase 1  (conv stacks): two 3-layer CNNs (4->12->12->12, 3x3, relu) computed
           as block-banded matmuls. Layout: partitions = (y_row_window x chan),
           free dim = x. The y-taps of each 3x3 conv live inside a banded lhsT
           (contract over (y_in, c)); the x-taps are 3 PSUM-accumulated matmuls
           over free-dim-shifted views. Strips of 8 output rows; the output
           grid drifts +1 row per layer so PSUM evictions always land at
           natural partitions; strip-to-strip halo rows move via tiny DMAs.
  Phase 2  (softmax green): E=exp(relu(w3)), i=relu(f3); selector matmuls
           reduce over channels-in-partitions giving g0num/g1num/den planes.
  Phase 2.5: rden=1/den; g0,g1; chroma c1=mosaic1-g0, c2=mosaic2-g1.
  Phase 3  (chroma 5x5 convs): in pixel-shuffled space each needed
           (conv, phase) output is a 12-tap stencil over (c1,c2) within a
           3x3 quad-space window -> same banded-matmul machinery (y-band in
           partitions, 3 x-passes), 6 outputs at once.
  Phase 4  (assembly): DVE writes with stride-2 free APs interleave quad
           planes into full-res rows; contiguous row DMAs to DRAM.

All conv matmuls run as float32r (full PE rate at N=512).
"""

import numpy as np

import concourse.bass as bass
import concourse.tile as tile
from concourse import bacc, mybir
from concourse.tile import add_dep_helper as _adh


def add_dep(frm, to, reason=""):
    _adh(frm.ins, to.ins, reason=reason)
from concourse.bass_utils import run_bass_kernel_spmd

F32 = mybir.dt.float32
import os
F32R = mybir.dt.float32 if os.environ.get("K_FP32") else mybir.dt.float32r
RELU = mybir.ActivationFunctionType.Relu
EXP = mybir.ActivationFunctionType.Exp

WIDTH = 12
HW = 512  # image H = W
NSTRIP = 65  # strips s = -1 .. 63, stride 8


# ---------------------------------------------------------------- host prep

def _band_lhsT(W, cin):
    """W: [12, cin, 3, 3] -> [3, 10*cin, 96] banded matrices (one per x-tap).

    lhsT_dx[(yi*cin + c), (yo*12 + oc)] = W[oc, c, yi - yo, dx]
    """
    K, M = 10 * cin, 8 * WIDTH
    out = np.zeros((3, K, M), np.float32)
    for dx in range(3):
        for yo in range(8):
            for dy in range(3):
                yi = yo + dy
                out[dx, yi * cin:(yi + 1) * cin, yo * WIDTH:(yo + 1) * WIDTH] = \
                    W[:, :, dy, dx].T
    return out


def _selectors():
    selA = np.zeros((96, 24), np.float32)  # applied to i*E
    selB = np.zeros((96, 24), np.float32)  # applied to E
    for yl in range(8):
        for c in range(WIDTH):
            p = yl * WIDTH + c
            if c < 6:
                selA[p, yl * 3 + 0] = 1.0
            else:
                selA[p, yl * 3 + 1] = 1.0
            selB[p, yl * 3 + 2] = 1.0
    return selA, selB


def _g_stencil(K5, py, px):
    """12-tap quad-space stencil of a 5x5 conv output at phase (py,px),
    over chroma channels c1 (phase (0,1)) and c2 (phase (1,0))."""
    G = np.zeros((2, 3, 3), np.float32)
    for cc, (qy, qx) in enumerate(((0, 1), (1, 0))):
        for dy in (-1, 0, 1):
            for dx in (-1, 0, 1):
                d5y = 2 * dy + 2 - py + qy
                d5x = 2 * dx + 2 - px + qx
                if 0 <= d5y < 5 and 0 <= d5x < 5:
                    G[cc, dy + 1, dx + 1] = K5[d5y, d5x]
    return G


def _chroma_lhsT(chw, cvw, cqw):
    """-> [3, 36, 96] banded matrices for the 6 (conv, phase) outputs.

    Output order o: 0 ch@(0,0), 1 ch@(1,1), 2 cv@(0,0), 3 cv@(1,1),
                    4 cq@(1,0), 5 cq@(0,1).
    """
    specs = [(chw, 0, 0), (chw, 1, 1), (cvw, 0, 0), (cvw, 1, 1),
             (cqw, 1, 0), (cqw, 0, 1)]
    out = np.zeros((3, 64, 96), np.float32)
    for o, (K5, py, px) in enumerate(specs):
        G = _g_stencil(np.asarray(K5)[0, 0], py, px)
        for dx in range(3):
            for yo in range(16):
                for dy in (-1, 0, 1):
                    yi = yo + dy + 1
                    for cc in range(2):
                        out[dx, cc * 32 + yi, yo * 6 + o] = G[cc, dy + 1, dx]
    return out


def _host_prep(inputs):
    mosaic = np.asarray(inputs["mosaic"], np.float32)  # [8,4,512,512]
    mospad = np.zeros((mosaic.shape[0], 4, 522, 514), np.float32)
    mospad[:, :, 8:520, 1:513] = mosaic
    w1 = np.stack([_band_lhsT(np.asarray(inputs["fw0"]), 4),
                   _band_lhsT(np.asarray(inputs["ww0"]), 4)])  # [2,3,40,96]
    w23 = np.stack([_band_lhsT(np.asarray(inputs["fw1"]), 12),
                    _band_lhsT(np.asarray(inputs["ww1"]), 12),
                    _band_lhsT(np.asarray(inputs["fw2"]), 12),
                    _band_lhsT(np.asarray(inputs["ww2"]), 12)])  # [4,3,120,96]
    selA, selB = _selectors()
    sel = np.stack([selA, selB])  # [2,96,24]
    w5 = _chroma_lhsT(inputs["chw"], inputs["cvw"], inputs["cqw"])  # [3,64,96]
    return mospad, {"w1": w1, "w23": w23, "sel": sel, "w5": w5}


# ---------------------------------------------------------------- kernel IR

def _dma_rows_to_plane(nc, plane, src_ap, y_start, nrows, clip=HW):
    """DMA nrows of src (row r -> image row y_start+r) into a [128, 4*512]
    plane laid out y -> (partition y%128, free (y//128)*512 + x).
    Splits at 128-partition boundaries, clips y to [0, clip)."""
    y0, y1 = max(y_start, 0), min(y_start + nrows, clip)
    while y0 < y1:
        run = min(y1 - y0, 128 - (y0 % 128))
        r0 = y0 - y_start
        p0 = y0 % 128
        f0 = (y0 // 128) * 512
        nc.sync.dma_start(plane[p0:p0 + run, f0:f0 + 512],
                          src_ap[r0:r0 + run, :])
        y0 += run


def build_kernel(tc, outs, ins, ctx):
    nc = tc.nc
    mospad, w1, w23, sel, w5 = (ins[k] for k in
                                ("mospad", "w1", "w23", "sel", "w5"))
    out = outs["out"]

    wp = ctx.enter_context(tc.tile_pool(name="weights", bufs=1))
    pp = ctx.enter_context(tc.tile_pool(name="planes", bufs=1))
    ps = ctx.enter_context(tc.tile_pool(name="ps", bufs=6, space="PSUM"))
    ps2 = ctx.enter_context(tc.tile_pool(name="ps2", bufs=2, space="PSUM"))
    pools = {}
    for tag in ("b0", "b1f", "b1w", "b2f", "b2w", "b3"):
        pools[tag] = ctx.enter_context(tc.tile_pool(name=f"p_{tag}", bufs=4))
    ph2 = ctx.enter_context(tc.tile_pool(name="ph2", bufs=3))
    qpp = ctx.enter_context(tc.tile_pool(name="qp", bufs=1))
    asmp = ctx.enter_context(tc.tile_pool(name="asm", bufs=4))

    # --- weights to SBUF
    w1_t = wp.tile([40, 6 * 96], F32R, tag="w1")
    for st in range(2):
        for dx in range(3):
            nc.sync.dma_start(w1_t[:, (st * 3 + dx) * 96:(st * 3 + dx + 1) * 96],
                              w1[st, dx])
    w23_t = wp.tile([120, 12 * 96], F32R, tag="w23")
    for ly in range(4):
        for dx in range(3):
            nc.sync.dma_start(
                w23_t[:, (ly * 3 + dx) * 96:(ly * 3 + dx + 1) * 96], w23[ly, dx])
    sel_t = wp.tile([96, 48], F32R, tag="sel")
    nc.sync.dma_start(sel_t[:, 0:24], sel[0])
    nc.sync.dma_start(sel_t[:, 24:48], sel[1])
    w5_t = wp.tile([64, 3 * 96], F32R, tag="w5")
    for dx in range(3):
        nc.sync.dma_start(w5_t[:, dx * 96:(dx + 1) * 96], w5[dx])

    # --- persistent planes [128, 2048]: y -> (y%128, (y//128)*512 + x)
    mos_p = []
    for c in range(4):
        p = pp.tile([128, 2048], F32, tag=f"mos{c}")
        for t in range(4):
            nc.sync.dma_start(p[:, t * 512:(t + 1) * 512],
                              mospad[c, 8 + t * 128:8 + (t + 1) * 128,
                                     1:513].bitcast(F32))
        mos_p.append(p)
    g3 = pp.tile([128, 3 * 2048], F32, tag="g3")
    g0n = g3[:, 0:2048]
    g1n = g3[:, 2048:4096]
    den = g3[:, 4096:6144]
    c1p = pp.tile([128, 2048], F32R, tag="c1")
    c2p = pp.tile([128, 2048], F32R, tag="c2")
    zt = pp.tile([96, 514], F32R, tag="zt")
    nc.gpsimd.memset(zt[:].bitcast(F32), 0.0)

    # --- phase 1+2 wavefront over strips
    b0_t, b1_t, b2_t = {}, {}, {}  # s -> tile handles; b1/b2: (s, stack)

    def load_b0(s):
        t = pools["b0"].tile([40, 514], F32R, tag="b0", name="b0")
        src = mospad[:, 8 * s + 8:8 * s + 18, :].transpose([1, 0, 2])
        d = nc.sync.dma_start(t[:], src)
        b0_t[s] = (t, [d])

    def conv_pass(rhs_tile, kdim, w_tile, wofs, deps=()):
        psum = ps.tile([96, 512], F32, tag="cps", name="cps")

        def w(dx):
            return w_tile[0:kdim, wofs + dx * 96:wofs + (dx + 1) * 96]

        r = rhs_tile[0:kdim, :]
        mms = [nc.tensor.matmul(psum[:], w(dx), r[:, dx:dx + 512],
                                start=(dx == 0), stop=(dx == 2))
               for dx in range(3)]
        for mm in mms:
            for dep in deps:
                add_dep(mm, dep, reason="rhs-ready")
        return psum

    def evict_relu(psum, store, s, tag, k):
        # strip rows m=0..7 hold y = 8s+k+m; rows outside [0,512) must be
        # exactly zero (conv zero-padding) or they leak into the next layer
        t = pools[tag].tile([120, 514], F32R, tag=tag, name=tag)
        a = nc.scalar.activation(t[0:96, 1:513], psum[:], RELU)
        z0 = nc.gpsimd.memset(t[0:96, 0:514:513].bitcast(F32), 0.0)
        add_dep(z0, a, reason="pad-cols")
        insts = [a, z0]
        if s == -1 and 8 - k > 0:
            z = nc.sync.dma_start(t[0:(8 - k) * 12, :], zt[0:(8 - k) * 12, :])
            add_dep(z, a, reason="zero-pad-rows")
            add_dep(z, z0, reason="zero-pad-rows")
            insts.append(z)
        if s == 63 and 8 - k < 8:
            z = nc.sync.dma_start(t[(8 - k) * 12:96, :], zt[0:k * 12, :])
            add_dep(z, a, reason="zero-pad-rows")
            add_dep(z, z0, reason="zero-pad-rows")
            insts.append(z)
        store[(s, tag)] = (t, insts)

    def halo(store, s, tag, eng):
        # store[(s,tag)][96:120] <- store[(s+1,tag)][0:24]  (rows y+8, y+9)
        dst, insts = store[(s, tag)]
        if (s + 1, tag) in store:
            d = eng.dma_start(dst[96:120, :], store[(s + 1, tag)][0][0:24, :])
        else:
            d = eng.dma_start(dst[96:120, :], zt[0:24, :])
        for i_ in insts:
            add_dep(d, i_, reason="halo-after-evict")

    def phase2(s, psf, psw):
        it = ph2.tile([96, 512], F32R, tag="i")
        et = ph2.tile([96, 512], F32R, tag="e")
        nc.scalar.activation(it[:], psf[:], RELU)
        nc.scalar.activation(et[:], psw[:], EXP)
        nc.vector.tensor_scalar_max(et[:], et[:], 1.0)
        nc.vector.tensor_mul(it[:], it[:], et[:])  # i*E in place
        p2 = ps2.tile([24, 512], F32, tag="p2")
        nc.tensor.matmul(p2[:], sel_t[:, 0:24],
                         it[:], start=True, stop=False)
        nc.tensor.matmul(p2[:], sel_t[:, 24:48],
                         et[:], start=False, stop=True)
        s2 = ph2.tile([24, 512], F32, tag="s2")
        nc.vector.tensor_copy(s2[:], p2[:])
        ys = 8 * s + 3
        ya, yb = max(ys, 0), min(ys + 8, HW)
        while ya < yb:
            run = min(yb - ya, 128 - (ya % 128))
            p0 = ya % 128
            dst = g3[p0:p0 + run, :].rearrange(
                "p (s c x) -> p s c x", s=3, c=4)[:, :, ya // 128, :]
            sv = s2[(ya - ys) * 3:(ya - ys + run) * 3, :]
            nc.gpsimd.dma_start(dst, sv)
            ya += run

    # --- phases 2.5/3/4 as chunked functions, interleaved into the wavefront
    asm_specs = [  # (ch, py, px, qp index or None, plane addend or None)
        (0, 0, 0, 0, mos_p[0]), (0, 0, 1, None, mos_p[1]),
        (0, 1, 0, 4, g1n), (0, 1, 1, 3, mos_p[3]),
        (1, 0, 0, None, mos_p[0]), (1, 0, 1, None, g0n),
        (1, 1, 0, None, g1n), (1, 1, 1, None, mos_p[3]),
        (2, 0, 0, 2, mos_p[0]), (2, 0, 1, 5, g0n),
        (2, 1, 0, None, mos_p[2]), (2, 1, 1, 1, mos_p[3]),
    ]
    qp6_h = {}

    def phase25(t):
        # green + chroma for y rows 128t..128t+127 (free chunk t of planes)
        cs = slice(t * 512, (t + 1) * 512)
        nc.vector.reciprocal(den[:, cs], den[:, cs])
        nc.vector.tensor_mul(g0n[:, cs], g0n[:, cs], den[:, cs])  # g0
        nc.vector.tensor_mul(g1n[:, cs], g1n[:, cs], den[:, cs])  # g1
        nc.vector.tensor_sub(c1p[:, cs], mos_p[1][:, cs], g0n[:, cs])
        nc.vector.tensor_sub(c2p[:, cs], mos_p[2][:, cs], g1n[:, cs])

    def phase3_strip(sq):
        h = sq // 16
        if h not in qp6_h:
            qp6_h[h] = qpp.tile([128, 6 * 1024], F32, tag="qp6",
                                name=f"qp6_{h}")
        qp6 = qp6_h[h]
        b3 = pools["b3"].tile([64, 514], F32R, tag="b3", name="b3")
        wrs = [nc.gpsimd.memset(b3[0:64, 0:514:513].bitcast(F32), 0.0)]
        y0 = 16 * sq - 1
        if y0 < 0:
            wrs.append(nc.gpsimd.memset(b3[0:1, :].bitcast(F32), 0.0))
            wrs.append(nc.gpsimd.memset(b3[32:33, :].bitcast(F32), 0.0))
        if y0 + 18 > HW:
            wrs.append(nc.sync.dma_start(b3[17:18, :], zt[0:1, :]))
            wrs.append(nc.sync.dma_start(b3[49:50, :], zt[0:1, :]))
        for cc, src_plane in ((0, c1p), (1, c2p)):
            ya, yb = max(y0, 0), min(y0 + 18, HW)
            while ya < yb:
                run = min(yb - ya, 128 - (ya % 128))
                d = nc.sync.dma_start(
                    b3[cc * 32 + ya - y0:cc * 32 + ya - y0 + run, 1:513],
                    src_plane[ya % 128:ya % 128 + run,
                              (ya // 128) * 512:(ya // 128) * 512 + 512])
                wrs.append(d)
                ya += run
        wrs.append(nc.gpsimd.dma_start(b3[18:32, :], zt[0:14, :]))
        wrs.append(nc.gpsimd.dma_start(b3[50:64, :], zt[0:14, :]))
        p3 = ps.tile([96, 512], F32, tag="cps", name="p3")
        mm3 = [nc.tensor.matmul(p3[:], w5_t[:, dx * 96:(dx + 1) * 96],
                                b3[0:64, dx:dx + 512],
                                start=(dx == 0), stop=(dx == 2))
               for dx in range(3)]
        for mm in mm3:
            for wr in wrs:
                add_dep(mm, wr, reason="b3-ready")
        s3 = ph2.tile([96, 512], F32, tag="s3")
        nc.vector.tensor_copy(s3[:], p3[:])
        yq = 16 * sq
        tlc = (yq // 128) - 2 * h  # 0 or 1: 512-chunk within the half
        dst = qp6[yq % 128:yq % 128 + 16, :].rearrange(
            "p (o c x) -> p o c x", o=6, c=2)[:, :, tlc, :]
        nc.sync.dma_start(dst, s3[:])

    def assemble_half(h):
        qp6 = qp6_h[h]
        for tl in range(2):
            t = 2 * h + tl
            for ch in range(3):
                for py in range(2):
                    a = asmp.tile([128, 1024], F32, tag="asm", name="asm")
                    prev = None
                    for (c_, py_, px, qo, addend) in asm_specs:
                        if c_ != ch or py_ != py:
                            continue
                        view = a[:].rearrange("p (x two) -> p two x",
                                              two=2)[:, px, :]
                        if qo is None:
                            src = addend[:, t * 512:(t + 1) * 512]
                            w_ = nc.scalar.copy(view, src)
                        else:
                            w_ = nc.vector.tensor_add(
                                view,
                                qp6[:, qo * 1024 + tl * 512:
                                    qo * 1024 + tl * 512 + 512],
                                addend[:, t * 512:(t + 1) * 512])
                        if prev is not None:
                            add_dep(w_, prev, reason="asm-interleave")
                        prev = w_
                    dst = out[ch].rearrange("(y two) x -> two y x", two=2)[
                        py, t * 128:(t + 1) * 128, :]
                    nc.sync.dma_start(dst, a[:])

    # interleave: after phase2(t3) finishes the last strip of plane-chunk t
    # (t3 == 16t+15), emit that chunk's green/chroma and the phase-3 strips
    # it unlocks; assembly of each half follows its last phase-3 strip.
    def emit_chunk(t):
        phase25(t)
        lo = max(0, 8 * t - 1)
        hi = min(32, 8 * t + 7 + (1 if t == 3 else 0))
        for sq in range(lo, hi):
            phase3_strip(sq)
            if sq == 15:
                assemble_half(0)
            if sq == 31:
                assemble_half(1)

    for i in range(NSTRIP + 4):
        s = i - 1  # L1 strip index
        if s <= 63:
            load_b0(s)
            for st, tag in ((0, "b1f"), (1, "b1w")):
                evict_relu(conv_pass(b0_t[s][0], 40, w1_t, st * 3 * 96,
                                     deps=b0_t[s][1]), b1_t, s, tag, 1)
            if s - 1 >= -1:
                b0_t.pop(s - 1, None)
        t2 = s - 2  # L2 strip index (skewed: halo source already evicted)
        if -1 <= t2 <= 63:
            halo(b1_t, t2, "b1f", nc.gpsimd)
            halo(b1_t, t2, "b1w", nc.sync)
            for st, (tag_in, tag_out) in enumerate((("b1f", "b2f"),
                                                    ("b1w", "b2w"))):
                evict_relu(conv_pass(b1_t[(t2, tag_in)][0], 120, w23_t,
                                     st * 3 * 96), b2_t, t2, tag_out, 2)
        t3 = s - 4  # L3 strip index (skewed)
        if -1 <= t3 <= 63:
            for tag in ("b2f", "b2w"):
                halo(b2_t, t3, tag, nc.sync)
            psf = conv_pass(b2_t[(t3, "b2f")][0], 120, w23_t, 2 * 3 * 96)
            psw = conv_pass(b2_t[(t3, "b2w")][0], 120, w23_t, 3 * 3 * 96)
            phase2(t3, psf, psw)
            for tag in ("b1f", "b1w"):
                b1_t.pop((t3, tag), None)
            if t3 - 1 >= -1:
                for tag in ("b2f", "b2w"):
                    b2_t.pop((t3 - 1, tag), None)
            if t3 in (15, 31, 47, 63):
                emit_chunk(t3 // 16)



_CACHE = {}


def _get_compiled():
    if "nc" in _CACHE:
        return _CACHE["nc"]
    nc = bacc.Bacc("TRN2", target_bir_lowering=False, debug=False,
                   enable_asserts=False)
    ins = {
        "mospad": nc.dram_tensor("mospad", [4, 522, 514], F32R,
                                 kind="ExternalInput").ap(),
        "w1": nc.dram_tensor("w1", [2, 3, 40, 96], F32R,
                             kind="ExternalInput").ap(),
        "w23": nc.dram_tensor("w23", [4, 3, 120, 96], F32R,
                              kind="ExternalInput").ap(),
        "sel": nc.dram_tensor("sel", [2, 96, 24], F32R,
                              kind="ExternalInput").ap(),
        "w5": nc.dram_tensor("w5", [3, 64, 96], F32R,
                             kind="ExternalInput").ap(),
    }
    outs = {"out": nc.dram_tensor("out", [3, 1024, 1024], F32,
                                  kind="ExternalOutput").ap()}
    from contextlib import ExitStack
    with tile.TileContext(nc) as tc, ExitStack() as ctx:
        build_kernel(tc, outs, ins, ctx)
    nc.compile()
    _CACHE["nc"] = nc
    return nc


def kernel(**inputs):
    nc = _get_compiled()
    mospad, shared = _host_prep(inputs)
    in_maps = []
    for b in range(8):
        m = {"mospad": np.ascontiguousarray(mospad[b])}
        m.update(shared)
        in_maps.append(m)
    res = run_bass_kernel_spmd(nc, in_maps, core_ids=list(range(8)))
    return np.stack([res.results[b]["out"] for b in range(8)])



# revision 98
# speedup vs baseline: 1.4415x; 1.4415x over previous
"""Trainium2 Bass kernel for the BasicQuadRGBV2 demosaic model.

Data-parallel over batch: 1 image per NeuronCore (8 cores).

Per-core dataflow (image [4,512,512] -> [3,1024,1024]):
  Phase 1  (conv stacks): two 3-layer CNNs (4->12->12->12, 3x3, relu) computed
           as block-banded matmuls. Layout: partitions = (y_row_window x chan),
           free dim = x. The y-taps of each 3x3 conv live inside a banded lhsT
           (contract over (y_in, c)); the x-taps are 3 PSUM-accumulated matmuls
           over free-dim-shifted views. Strips of 8 output rows; the output
           grid drifts +1 row per layer so PSUM evictions always land at
           natural partitions. The f- and w-stacks of one layer share a fused
           2-bank PSUM pair and one eviction; strip-to-strip halo rows move via
           partition-shifted engine copies (96<-0), not DMA.
  Phase 2  (softmax green): E=max(exp(w3),1)=exp(relu(w3)), i=relu(f3);
           selector matmuls (M=24, 3 slots/row) reduce over channels; a DVE
           eviction + one small DMA per strip scatter g0num/g1num/den planes.
  Phase 2.5: rden=1/den; g0,g1; chroma c1|c2 interleaved in one plane;
           runs per 32-row quarter as soon as its g3 rows are scattered, and
           phase-3 strips are drip-fed ~13 iterations later so PE never
           head-of-line blocks on the gather chain.
  Phase 3  (chroma 5x5 convs): in pixel-shuffled space each needed
           (conv, phase) output is a 12-tap stencil over (c1,c2) within a
           3x3 quad-space window -> same banded-matmul machinery; b3 rhs tiles
           are persistent pre-zeroed buffers filled by one gather DMA each.
  Phase 4  (assembly): DVE/ACT writes with stride-2 free APs interleave quad
           planes into [128,2048] tiles; one contiguous 1MiB DMA per (ch,qtr).

All conv matmuls run as float32r (full PE rate at N=512).
"""

import numpy as np

import concourse.bass as bass
import concourse.tile as tile
from concourse import bacc, mybir
from concourse.tile import add_dep_helper as _adh


def add_dep(frm, to, reason=""):
    _adh(frm.ins, to.ins, reason=reason)
from concourse.bass_utils import run_bass_kernel_spmd

F32 = mybir.dt.float32
import os
F32R = mybir.dt.float32 if os.environ.get("K_FP32") else mybir.dt.float32r
RELU = mybir.ActivationFunctionType.Relu
EXP = mybir.ActivationFunctionType.Exp

WIDTH = 12
HW = 512  # image H = W
NSTRIP = 65  # strips s = -1 .. 63, stride 8


# ---------------------------------------------------------------- host prep

def _band_lhsT(W, cin):
    """W: [12, cin, 3, 3] -> [3, 10*cin, 96] banded matrices (one per x-tap).

    lhsT_dx[(yi*cin + c), (yo*12 + oc)] = W[oc, c, yi - yo, dx]
    """
    K, M = 10 * cin, 8 * WIDTH
    out = np.zeros((3, K, M), np.float32)
    for dx in range(3):
        for yo in range(8):
            for dy in range(3):
                yi = yo + dy
                out[dx, yi * cin:(yi + 1) * cin, yo * WIDTH:(yo + 1) * WIDTH] = \
                    W[:, :, dy, dx].T
    return out


def _selectors():
    # M=24: out partition = 3*row + v; v: 0=g0num, 1=g1num, 2=den
    selA = np.zeros((96, 24), np.float32)  # applied to i*E
    selB = np.zeros((96, 24), np.float32)  # applied to E
    for yl in range(8):
        for c in range(WIDTH):
            p = yl * WIDTH + c
            selA[p, yl * 3 + (0 if c < 6 else 1)] = 1.0
            selB[p, yl * 3 + 2] = 1.0
    return selA, selB


def _g_stencil(K5, py, px):
    """12-tap quad-space stencil of a 5x5 conv output at phase (py,px),
    over chroma channels c1 (phase (0,1)) and c2 (phase (1,0))."""
    G = np.zeros((2, 3, 3), np.float32)
    for cc, (qy, qx) in enumerate(((0, 1), (1, 0))):
        for dy in (-1, 0, 1):
            for dx in (-1, 0, 1):
                d5y = 2 * dy + 2 - py + qy
                d5x = 2 * dx + 2 - px + qx
                if 0 <= d5y < 5 and 0 <= d5x < 5:
                    G[cc, dy + 1, dx + 1] = K5[d5y, d5x]
    return G


def _chroma_lhsT(chw, cvw, cqw):
    """-> [3, 64, 96] banded matrices for the 6 (conv, phase) outputs.

    Output order o: 0 ch@(0,0), 1 ch@(1,1), 2 cv@(0,0), 3 cv@(1,1),
                    4 cq@(1,0), 5 cq@(0,1).
    """
    specs = [(chw, 0, 0), (chw, 1, 1), (cvw, 0, 0), (cvw, 1, 1),
             (cqw, 1, 0), (cqw, 0, 1)]
    out = np.zeros((3, 64, 96), np.float32)
    for o, (K5, py, px) in enumerate(specs):
        G = _g_stencil(np.asarray(K5)[0, 0], py, px)
        for dx in range(3):
            for yo in range(16):
                for dy in (-1, 0, 1):
                    yi = yo + dy + 1
                    for cc in range(2):
                        out[dx, cc * 32 + yi, yo * 6 + o] = G[cc, dy + 1, dx]
    return out


# wpack column layout
W1_OFS = 0        # [40 | +64..104 dup, 576)  f-stack cols 0:288, w-stack 288:576
W23_OFS = 576     # [120, 1152)
SELA_OFS = 1728   # [96, 24)
SELB_OFS = 1752   # [96, 24)
W5_OFS = 1776     # [64, 288)
WPACK_W = 2064


def _host_prep(inputs):
    mosaic = np.asarray(inputs["mosaic"], np.float32)  # [8,4,512,512]
    mospad = np.zeros((mosaic.shape[0], 4, 522, 514), np.float32)
    mospad[:, :, 8:520, 1:513] = mosaic
    w1f = _band_lhsT(np.asarray(inputs["fw0"]), 4)  # [3,40,96]
    w1w = _band_lhsT(np.asarray(inputs["ww0"]), 4)
    w23 = np.stack([_band_lhsT(np.asarray(inputs["fw1"]), 12),
                    _band_lhsT(np.asarray(inputs["ww1"]), 12),
                    _band_lhsT(np.asarray(inputs["fw2"]), 12),
                    _band_lhsT(np.asarray(inputs["ww2"]), 12)])  # [4,3,120,96]
    selA, selB = _selectors()
    w5 = _chroma_lhsT(inputs["chw"], inputs["cvw"], inputs["cqw"])  # [3,64,96]
    wpack = np.zeros((128, WPACK_W), np.float32)
    # L1 f-stack at partitions 0:40 cols 0:288; w-stack dup at partitions
    # 64:104 cols 288:576 (PE row-tiling: two concurrent K=40 matmuls)
    wpack[0:40, 0:288] = w1f.transpose(1, 0, 2).reshape(40, 288)
    wpack[64:104, 288:576] = w1w.transpose(1, 0, 2).reshape(40, 288)
    wpack[0:120, W23_OFS:W23_OFS + 1152] = w23.transpose(0, 2, 1, 3).reshape(
        4, 120, 288).transpose(1, 0, 2).reshape(120, 1152)
    wpack[0:96, SELA_OFS:SELA_OFS + 24] = selA
    wpack[0:96, SELB_OFS:SELB_OFS + 24] = selB
    wpack[0:64, W5_OFS:W5_OFS + 288] = w5.transpose(1, 0, 2).reshape(64, 288)
    return mospad, {"wpack": wpack}


# ---------------------------------------------------------------- kernel IR

def build_kernel(tc, outs, ins, ctx):
    nc = tc.nc
    mospad, wpack = ins["mospad"], ins["wpack"]
    out = outs["out"]

    wp = ctx.enter_context(tc.tile_pool(name="weights", bufs=1))
    pp = ctx.enter_context(tc.tile_pool(name="planes", bufs=1))
    ps = ctx.enter_context(tc.tile_pool(name="ps", bufs=3, space="PSUM"))
    # 8 PSUM banks total: 3x2 fused conv pairs + 1 phase-2 + 1 phase-3
    ps23 = ctx.enter_context(tc.tile_pool(name="ps23", bufs=2, space="PSUM"))
    pools = {}
    for tag in ("b1", "b2"):
        pools[tag] = ctx.enter_context(tc.tile_pool(name=f"p_{tag}", bufs=4))
    b0p = ctx.enter_context(tc.tile_pool(name="p_b0", bufs=4))
    ph2 = ctx.enter_context(tc.tile_pool(name="ph2", bufs=4))
    stgp = ctx.enter_context(tc.tile_pool(name="stg", bufs=3))
    b3p = ctx.enter_context(tc.tile_pool(name="b3", bufs=1))
    qpp = ctx.enter_context(tc.tile_pool(name="qp", bufs=2))
    asmp = ctx.enter_context(tc.tile_pool(name="asm", bufs=2))

    # --- weights to SBUF: one packed DMA
    wpack_t = wp.tile([128, WPACK_W], F32R, tag="wpack")
    nc.sync.dma_start(wpack_t[:], wpack)
    w23_t = wpack_t[0:120, W23_OFS:W23_OFS + 1152]
    selA_t = wpack_t[0:96, SELA_OFS:SELA_OFS + 24]
    selB_t = wpack_t[0:96, SELB_OFS:SELB_OFS + 24]
    w5_t = wpack_t[0:64, W5_OFS:W5_OFS + 288]

    # --- persistent planes [128, 2048]: y -> (y%128, (y//128)*512 + x)
    # (tiles created now; DMAs deferred into the strip loop so the first
    # b0 loads aren't stuck behind 4 MiB of plane traffic on SP)
    mos_p = [pp.tile([128, 2048], F32, tag=f"mos{c}", name=f"mos{c}")
             for c in range(4)]

    def load_mos_plane(c):
        nc.sync.dma_start(
            mos_p[c][:].rearrange("p (t x) -> p t x", t=4),
            mospad[c, 8:520, 1:513].bitcast(F32).rearrange(
                "(t p) x -> p t x", p=128))
    g3 = pp.tile([128, 3 * 2048], F32, tag="g3")
    g0n = g3[:, 0:2048]
    g1n = g3[:, 2048:4096]
    den = g3[:, 4096:6144]
    # chroma c1|c2 interleaved: free = cc*2048 + chunk*512 + x
    ccp = pp.tile([128, 4096], F32R, tag="ccp")
    zt = pp.tile([96, 1028], F32R, tag="zt")
    nc.gpsimd.memset(zt[:].bitcast(F32), 0.0)

    # --- phase 1+2 wavefront over strips
    b0_t, bt = {}, {}  # s -> tile handles; bt: (s, tag)

    def load_b0(s):
        # dual copy at partitions 0:40 and 64:104 for L1 PE row-tiling
        t = b0p.tile([128, 514], F32R, tag="b0", name="b0")
        src = mospad[:, 8 * s + 8:8 * s + 18, :].transpose([1, 0, 2])
        d0 = nc.sync.dma_start(t[0:40, :], src)
        d1 = nc.gpsimd.tensor_copy(t[64:104, :], t[0:40, :])
        add_dep(d1, d0, reason="b0-dup")
        b0_t[s] = (t, [d0, d1])

    def conv_l1(s):
        # two concurrent K=40 row-tiles: f at rows 0:40, w at rows 64:104
        t, deps = b0_t[s]
        psum = ps.tile([96, 1024], F32, tag="cps", name="cps")
        for st in range(2):
            for dx in range(3):
                mm = nc.tensor.matmul(
                    psum[:, st * 512:(st + 1) * 512],
                    wpack_t[64 * st:64 * st + 40,
                            st * 288 + dx * 96:st * 288 + (dx + 1) * 96],
                    t[64 * st:64 * st + 40, dx:dx + 512],
                    start=(dx == 0), stop=(dx == 2))
                for dep in deps:
                    add_dep(mm, dep, reason="rhs-ready")
        return psum

    def conv_pair(tag_in, s, ly_f, ly_w):
        # one layer for both stacks from a fused input tile [120, 1028]
        t, insts = bt[(s, tag_in)]
        psum = ps.tile([96, 1024], F32, tag="cps", name="cps")
        for st, ly in ((0, ly_f), (1, ly_w)):
            for dx in range(3):
                mm = nc.tensor.matmul(
                    psum[:, st * 512:(st + 1) * 512],
                    w23_t[:, ly * 288 + dx * 96:ly * 288 + (dx + 1) * 96],
                    t[0:120, st * 514 + dx:st * 514 + dx + 512],
                    start=(dx == 0), stop=(dx == 2))
                for dep in insts:
                    add_dep(mm, dep, reason="rhs-ready")
        return psum

    def evict_pair(psum, s, tag, k):
        # strip rows m=0..7 hold y = 8s+k+m; rows outside [0,512) must be
        # exactly zero (conv zero-padding) or they leak into the next layer
        t = pools[tag].tile([120, 1028], F32R, tag=tag, name=tag)
        tv = t[0:96, :].rearrange("p (st x) -> p st x", st=2)
        a = nc.scalar.activation(
            tv[:, :, 1:513],
            psum[:].rearrange("p (st x) -> p st x", st=2), RELU)
        z0 = nc.gpsimd.memset(tv[:, :, 0:514:513].bitcast(F32), 0.0)
        add_dep(z0, a, reason="pad-cols")
        insts = [a, z0]
        if s == -1:
            z = nc.gpsimd.memset(t[0:(8 - k) * 12, :].bitcast(F32), 0.0)
            add_dep(z, a, reason="zero-pad-rows")
            add_dep(z, z0, reason="zero-pad-rows")
            insts.append(z)
        if s == 63:
            z = nc.sync.dma_start(t[(8 - k) * 12:96, :], zt[0:k * 12, :])
            add_dep(z, a, reason="zero-pad-rows")
            add_dep(z, z0, reason="zero-pad-rows")
            insts.append(z)
        bt[(s, tag)] = (t, insts)

    def halo(s, tag):
        # bt[(s,tag)][96:120] <- bt[(s+1,tag)][0:24]  (rows y+8, y+9)
        # partition-shifted engine copy (bases 96 and 0 are 32-aligned)
        dst, insts = bt[(s, tag)]
        eng = nc.vector.tensor_copy
        if (s + 1, tag) in bt:
            src_t, src_insts = bt[(s + 1, tag)]
            d = eng(dst[96:120, :], src_t[0:24, :])
            for i_ in src_insts:
                add_dep(d, i_, reason="halo-src-ready")
        else:
            d = nc.gpsimd.memset(dst[96:120, :].bitcast(F32), 0.0)
        for i_ in insts:
            add_dep(d, i_, reason="halo-after-evict")

    def phase2(s, pair):
        it = ph2.tile([96, 512], F32R, tag="i")
        et = ph2.tile([96, 512], F32R, tag="e")
        nc.scalar.activation(it[:], pair[:, 0:512], RELU)
        nc.scalar.activation(et[:], pair[:, 512:1024], EXP)
        # max(exp(x), 1) == exp(relu(x)) -- this IS the w-stack's last relu
        nc.vector.tensor_scalar_max(et[:], et[:], 1.0)
        nc.vector.tensor_mul(it[:], it[:], et[:])  # i*E in place
        p2 = ps23.tile([24, 512], F32, tag="p2x", name="p2", bufs=1)[0:24, :]
        nc.tensor.matmul(p2[:], selA_t, it[:], start=True, stop=False)
        nc.tensor.matmul(p2[:], selB_t, et[:], start=False, stop=True)
        s2 = stgp.tile([24, 512], F32, tag="stg", name="stg")
        nc.vector.tensor_copy(s2[:], p2[:])
        ys = 8 * s + 3
        ya, yb = max(ys, 0), min(ys + 8, HW)
        while ya < yb:
            run = min(yb - ya, 128 - (ya % 128))
            p0 = ya % 128
            dst = g3[p0:p0 + run, :].rearrange(
                "p (v c x) -> p v c x", v=3, c=4)[:, :, ya // 128, :]
            sv = s2[(ya - ys) * 3:(ya - ys + run) * 3, :]
            nc.sync.dma_start(dst, sv)
            ya += run

    # --- phases 2.5/3/4, interleaved into the wavefront
    asm_specs = [  # (ch, py, px, qp index or None, plane addend or None)
        (0, 0, 0, 0, mos_p[0]), (0, 0, 1, None, mos_p[1]),
        (0, 1, 0, 4, g1n), (0, 1, 1, 3, mos_p[3]),
        (1, 0, 0, None, mos_p[0]), (1, 0, 1, None, g0n),
        (1, 1, 0, None, g1n), (1, 1, 1, None, mos_p[3]),
        (2, 0, 0, 2, mos_p[0]), (2, 0, 1, 5, g0n),
        (2, 1, 0, None, mos_p[2]), (2, 1, 1, 1, mos_p[3]),
    ]
    qp6_q = {}

    # persistent pre-zeroed b3 rhs buffers: rows 18..31 / 50..63 and pad
    # cols 0,513 stay zero forever; gathers only write rows 0..17/32..49,
    # cols 1:513.  sq==0 leaves rows 0,32 pristine (buf first use); sq==31
    # gets its own buf 4 so rows 17,49 stay pristine.
    b3_bufs = []
    for i in range(5):
        t = b3p.tile([64, 514], F32R, tag=f"b3_{i}", name=f"b3_{i}")
        nc.gpsimd.memset(t[:].bitcast(F32), 0.0)
        b3_bufs.append(t)

    def phase25(t, q):
        # green + chroma for y rows 128t+32q .. 128t+32q+31 (partition quarter
        # q of free chunk t); complete right after strip 16t+4q+3's scatter
        f0 = t * 512
        pr = slice(32 * q, 32 * q + 32)

        def g3v(v):
            return g3[pr, v * 2048 + f0:v * 2048 + f0 + 512]

        nc.vector.reciprocal(g3v(2), g3v(2))
        nc.vector.tensor_mul(g3v(0), g3v(0), g3v(2))  # g0
        nc.vector.tensor_mul(g3v(1), g3v(1), g3v(2))  # g1
        nc.vector.tensor_sub(ccp[pr, f0:f0 + 512],
                             mos_p[1][pr, f0:f0 + 512], g3v(0))
        nc.vector.tensor_sub(ccp[pr, 2048 + f0:2048 + f0 + 512],
                             mos_p[2][pr, f0:f0 + 512], g3v(1))

    def phase3_strip(sq):
        qt = sq // 8
        if qt not in qp6_q:
            qp6_q[qt] = qpp.tile([128, 6 * 512], F32, tag="qp6",
                                 name=f"qp6_{qt}")
        qp6 = qp6_q[qt]
        b3 = b3_bufs[4 if sq == 31 else sq % 4]
        y0 = 16 * sq - 1
        ya, yb = max(y0, 0), min(y0 + 18, HW)
        wrs = []
        while ya < yb:
            run = min(yb - ya, 128 - (ya % 128))
            p0 = ya % 128
            tlc = ya // 128
            for cc in range(2):
                eng = nc.sync if cc == 0 else nc.scalar
                d = eng.dma_start(
                    b3[cc * 32 + ya - y0:cc * 32 + ya - y0 + run, 1:513],
                    ccp[p0:p0 + run,
                        cc * 2048 + tlc * 512:cc * 2048 + tlc * 512 + 512])
                wrs.append(d)
            ya += run
        p3 = ps23.tile([96, 512], F32, tag="p23", name="p3", bufs=1)
        mm3 = [nc.tensor.matmul(p3[:], w5_t[:, dx * 96:(dx + 1) * 96],
                                b3[0:64, dx:dx + 512],
                                start=(dx == 0), stop=(dx == 2))
               for dx in range(3)]
        for mm in mm3:
            for wr in wrs:
                add_dep(mm, wr, reason="b3-ready")
        s3 = ph2.tile([96, 512], F32, tag="s3")
        nc.vector.tensor_copy(s3[:], p3[:])
        yq = 16 * sq
        qeng = (nc.gpsimd, nc.sync, nc.scalar)[sq % 3] if sq >= 29 \
            else nc.gpsimd
        qeng.dma_start(
            qp6[yq % 128:yq % 128 + 16, :].rearrange("p (o x) -> p o x", o=6),
            s3[:])

    def assemble_quarter(t):
        qp6 = qp6_q.pop(t)
        for ch in range(3):
            a = asmp.tile([128, 2048], F32, tag="asm", name="asm")
            prev = None
            for (c_, py, px, qo, addend) in asm_specs:
                if c_ != ch:
                    continue
                view = a[:].rearrange("p (py x two) -> p py two x",
                                      py=2, two=2)[:, py, px, :]
                if qo is None:
                    src = addend[:, t * 512:(t + 1) * 512]
                    nc.gpsimd.tensor_copy(view, src)
                else:
                    nc.vector.tensor_add(
                        view, qp6[:, qo * 512:qo * 512 + 512],
                        addend[:, t * 512:(t + 1) * 512])
            # out[ch] rows 256t..256t+255 are exactly a[:] flattened
            dst = out[ch][256 * t:256 * (t + 1), :].rearrange(
                "(p f) x -> p (f x)", f=2)
            nc.sync.dma_start(dst, a[:])

    # interleave: after phase2(t3) finishes the last strip of plane-chunk t
    # (t3 == 16t+15), run that chunk's green/chroma and enqueue the phase-3
    # strips it unlocks; they're then drip-fed one per strip iteration so PE
    # always has conv matmuls between the gather-dependent phase-3 matmuls.
    pending = []
    PDELAY = int(os.environ.get("K_PDELAY", "13"))
    PCLAMP = int(os.environ.get("K_PCLAMP", "64"))

    next_sq = [0]
    sched = [0]

    def emit_quarter_chunk(t, q, ready_i):
        phase25(t, q)
        # ccp rows <= 128t+32q+31 now ready; phase-3 strip sq needs rows
        # <= 16sq+16, so strips up to 8t+2q+1 are unlocked
        hi = min(8 * t + 2 * q + 1, 31)
        for sq in range(next_sq[0], hi + 1):
            r = max(ready_i, sched[0] + 1)
            sched[0] = r
            pending.append((r, "sq", sq))
            if sq % 8 == 7:
                pending.append((r, "asm", sq // 8))
        next_sq[0] = max(next_sq[0], hi + 1)

    def run_pending(now_i, n):
        while pending and n > 0 and pending[0][0] <= now_i:
            _, kind, arg = pending.pop(0)
            if kind == "sq":
                phase3_strip(arg)
            else:
                assemble_quarter(arg)
            n -= 1

    load_b0(-1)
    for i in range(NSTRIP + 4):
        s = i - 1  # L1 strip index
        if s <= 63:
            if s + 1 <= 63:
                load_b0(s + 1)  # prefetch next strip's input
            evict_pair(conv_l1(s), s, "b1", 1)
            if s - 1 >= -1:
                b0_t.pop(s - 1, None)
        if i == 3:
            for c in range(4):
                load_mos_plane(c)
        t2 = s - 2  # L2 strip index (skewed: halo source already evicted)
        if -1 <= t2 <= 63:
            halo(t2, "b1")
            evict_pair(conv_pair("b1", t2, 0, 1), t2, "b2", 2)
        t3 = s - 4  # L3 strip index (skewed)
        if -1 <= t3 <= 63:
            halo(t3, "b2")
            phase2(t3, conv_pair("b2", t3, 2, 3))
            bt.pop((t3, "b1"), None)
            if t3 - 1 >= -1:
                bt.pop((t3 - 1, "b2"), None)
            if t3 >= 3 and t3 % 4 == 3:
                # items become runnable PDELAY iterations after their emit,
                # but clamp near the loop end so late-chunk items interleave
                # with the L1-drained final iterations instead of bunching
                # into the serial post-loop drain
                emit_quarter_chunk(t3 // 16, (t3 % 16) // 4,
                                   max(min(i + PDELAY, PCLAMP), i + 2))
        run_pending(i, 1)
    run_pending(10 ** 9, len(pending))


_CACHE = {}


def _get_compiled():
    if "nc" in _CACHE:
        return _CACHE["nc"]
    nc = bacc.Bacc("TRN2", target_bir_lowering=False, debug=False,
                   enable_asserts=False)
    ins = {
        "mospad": nc.dram_tensor("mospad", [4, 522, 514], F32R,
                                 kind="ExternalInput").ap(),
        "wpack": nc.dram_tensor("wpack", [128, WPACK_W], F32R,
                                kind="ExternalInput").ap(),
    }
    outs = {"out": nc.dram_tensor("out", [3, 1024, 1024], F32,
                                  kind="ExternalOutput").ap()}
    from contextlib import ExitStack
    with tile.TileContext(nc) as tc, ExitStack() as ctx:
        build_kernel(tc, outs, ins, ctx)
    nc.compile()
    _CACHE["nc"] = nc
    return nc


def kernel(**inputs):
    nc = _get_compiled()
    mospad, shared = _host_prep(inputs)
    in_maps = []
    for b in range(8):
        m = {"mospad": np.ascontiguousarray(mospad[b])}
        m.update(shared)
        in_maps.append(m)
    res = run_bass_kernel_spmd(nc, in_maps, core_ids=list(range(8)))
    return np.stack([res.results[b]["out"] for b in range(8)])


# revision 143
# speedup vs baseline: 1.7234x; 1.1956x over previous
"""Trainium2 Bass kernel for the BasicQuadRGBV2 demosaic model.

Data-parallel over batch: 1 image per NeuronCore (8 cores).

Per-core dataflow (image [4,512,512] -> [3,1024,1024]):
  Phase 1  (conv stacks): two 3-layer CNNs (4->12->12->12, 3x3, relu) computed
           as block-banded matmuls. Layout: partitions = (y_row_window x chan),
           free dim = x. The y-taps of each 3x3 conv live inside a banded lhsT
           (contract over (y_in, c)); the x-taps are 3 PSUM-accumulated matmuls
           over free-dim-shifted views. Strips of 8 output rows; the output
           grid drifts +1 row per layer so PSUM evictions always land at
           natural partitions. The f- and w-stacks of one layer share a fused
           2-bank PSUM pair and one eviction; strip-to-strip halo rows move via
           partition-shifted engine copies (96<-0), not DMA.
  Phase 2  (softmax green): E=max(exp(w3),1)=exp(relu(w3)), i=relu(f3);
           selector matmuls (M=24, 3 slots/row) reduce over channels; a DVE
           eviction + one small DMA per strip scatter g0num/g1num/den planes.
  Phase 2.5: rden=1/den; g0,g1; chroma c1|c2 interleaved in one plane;
           runs per 32-row quarter as soon as its g3 rows are scattered, and
           phase-3 strips are drip-fed ~13 iterations later so PE never
           head-of-line blocks on the gather chain.
  Phase 3  (chroma 5x5 convs): in pixel-shuffled space each needed
           (conv, phase) output is a 12-tap stencil over (c1,c2) within a
           3x3 quad-space window -> same banded-matmul machinery; b3 rhs tiles
           are persistent pre-zeroed buffers filled by one gather DMA each.
  Phase 4  (assembly): DVE/ACT writes with stride-2 free APs interleave quad
           planes into [128,2048] tiles; one contiguous 1MiB DMA per (ch,qtr).

All conv matmuls run as float32r (full PE rate at N=512).
"""

import numpy as np

import concourse.bass as bass
import concourse.tile as tile
from concourse import bacc, mybir
from concourse.tile import add_dep_helper as _adh


def add_dep(frm, to, reason=""):
    _adh(frm.ins, to.ins, reason=reason)
from concourse.bass_utils import run_bass_kernel_spmd

F32 = mybir.dt.float32
import os
F32R = mybir.dt.float32 if os.environ.get("K_FP32") else mybir.dt.float32r
RELU = mybir.ActivationFunctionType.Relu
EXP = mybir.ActivationFunctionType.Exp

WIDTH = 12
HW = 512  # image H = W
NSTRIP = 65  # strips s = -1 .. 63, stride 8


# ---------------------------------------------------------------- host prep

def _band_lhsT(W, cin):
    """W: [12, cin, 3, 3] -> [3, 10*cin, 96] banded matrices (one per x-tap).

    lhsT_dx[(yi*cin + c), (yo*12 + oc)] = W[oc, c, yi - yo, dx]
    """
    K, M = 10 * cin, 8 * WIDTH
    out = np.zeros((3, K, M), np.float32)
    for dx in range(3):
        for yo in range(8):
            for dy in range(3):
                yi = yo + dy
                out[dx, yi * cin:(yi + 1) * cin, yo * WIDTH:(yo + 1) * WIDTH] = \
                    W[:, :, dy, dx].T
    return out


def _selectors():
    # M=24: out partition = 3*row + v; v: 0=g0num, 1=g1num, 2=den
    selA = np.zeros((96, 24), np.float32)  # applied to i*E
    selB = np.zeros((96, 24), np.float32)  # applied to E
    for yl in range(8):
        for c in range(WIDTH):
            p = yl * WIDTH + c
            selA[p, yl * 3 + (0 if c < 6 else 1)] = 1.0
            selB[p, yl * 3 + 2] = 1.0
    return selA, selB


def _g_stencil(K5, py, px):
    """12-tap quad-space stencil of a 5x5 conv output at phase (py,px),
    over chroma channels c1 (phase (0,1)) and c2 (phase (1,0))."""
    G = np.zeros((2, 3, 3), np.float32)
    for cc, (qy, qx) in enumerate(((0, 1), (1, 0))):
        for dy in (-1, 0, 1):
            for dx in (-1, 0, 1):
                d5y = 2 * dy + 2 - py + qy
                d5x = 2 * dx + 2 - px + qx
                if 0 <= d5y < 5 and 0 <= d5x < 5:
                    G[cc, dy + 1, dx + 1] = K5[d5y, d5x]
    return G


def _chroma_lhsT(chw, cvw, cqw):
    """-> [3, 64, 96] banded matrices for the 6 (conv, phase) outputs.

    Output order o: 0 ch@(0,0), 1 ch@(1,1), 2 cv@(0,0), 3 cv@(1,1),
                    4 cq@(1,0), 5 cq@(0,1).
    """
    specs = [(chw, 0, 0), (chw, 1, 1), (cvw, 0, 0), (cvw, 1, 1),
             (cqw, 1, 0), (cqw, 0, 1)]
    out = np.zeros((3, 64, 96), np.float32)
    for o, (K5, py, px) in enumerate(specs):
        G = _g_stencil(np.asarray(K5)[0, 0], py, px)
        for dx in range(3):
            for yo in range(16):
                for dy in (-1, 0, 1):
                    yi = yo + dy + 1
                    for cc in range(2):
                        out[dx, cc * 32 + yi, yo * 6 + o] = G[cc, dy + 1, dx]
    return out


# wpack column layout
W1_OFS = 0        # [40 | +64..104 dup, 576)  f-stack cols 0:288, w-stack 288:576
W23_OFS = 576     # [120, 1152)
SELA_OFS = 1728   # [96, 24)
SELB_OFS = 1752   # [96, 24)
W5_OFS = 1776     # [64, 288)
WPACK_W = 2064


def _host_prep(inputs):
    mosaic = np.asarray(inputs["mosaic"], np.float32)  # [8,4,512,512]
    mospad = np.zeros((mosaic.shape[0], 4, 522, 514), np.float32)
    mospad[:, :, 8:520, 1:513] = mosaic
    w1f = _band_lhsT(np.asarray(inputs["fw0"]), 4)  # [3,40,96]
    w1w = _band_lhsT(np.asarray(inputs["ww0"]), 4)
    w23 = np.stack([_band_lhsT(np.asarray(inputs["fw1"]), 12),
                    _band_lhsT(np.asarray(inputs["ww1"]), 12),
                    _band_lhsT(np.asarray(inputs["fw2"]), 12),
                    _band_lhsT(np.asarray(inputs["ww2"]), 12)])  # [4,3,120,96]
    selA, selB = _selectors()
    w5 = _chroma_lhsT(inputs["chw"], inputs["cvw"], inputs["cqw"])  # [3,64,96]
    wpack = np.zeros((128, WPACK_W), np.float32)
    # L1 f-stack at partitions 0:40 cols 0:288; w-stack dup at partitions
    # 64:104 cols 288:576 (PE row-tiling: two concurrent K=40 matmuls)
    wpack[0:40, 0:288] = w1f.transpose(1, 0, 2).reshape(40, 288)
    wpack[64:104, 288:576] = w1w.transpose(1, 0, 2).reshape(40, 288)
    wpack[0:120, W23_OFS:W23_OFS + 1152] = w23.transpose(0, 2, 1, 3).reshape(
        4, 120, 288).transpose(1, 0, 2).reshape(120, 1152)
    wpack[0:96, SELA_OFS:SELA_OFS + 24] = selA
    wpack[0:96, SELB_OFS:SELB_OFS + 24] = selB
    wpack[0:64, W5_OFS:W5_OFS + 288] = w5.transpose(1, 0, 2).reshape(64, 288)
    return mospad, {"wpack": wpack}


# ---------------------------------------------------------------- kernel IR

def build_kernel(tc, outs, ins, ctx):
    nc = tc.nc
    mospad, wpack = ins["mospad"], ins["wpack"]
    out = outs["out"]

    wp = ctx.enter_context(tc.tile_pool(name="weights", bufs=1))
    pp = ctx.enter_context(tc.tile_pool(name="planes", bufs=1))
    ps = ctx.enter_context(tc.tile_pool(name="ps", bufs=3, space="PSUM"))
    # 8 PSUM banks total: 3x2 fused conv pairs + 1 phase-2 + 1 phase-3
    ps23 = ctx.enter_context(tc.tile_pool(name="ps23", bufs=2, space="PSUM"))
    pools = {}
    for tag in ("b1", "b2"):
        pools[tag] = ctx.enter_context(tc.tile_pool(name=f"p_{tag}", bufs=4))
    b0p = ctx.enter_context(tc.tile_pool(name="p_b0", bufs=4))
    ph2 = ctx.enter_context(tc.tile_pool(name="ph2", bufs=4))
    stgp = ctx.enter_context(tc.tile_pool(name="stg", bufs=3))
    b3p = ctx.enter_context(tc.tile_pool(name="b3", bufs=1))
    qpp = ctx.enter_context(tc.tile_pool(name="qp", bufs=2))
    asmp = ctx.enter_context(tc.tile_pool(name="asm", bufs=2))

    # --- weights to SBUF: one packed DMA
    wpack_t = wp.tile([128, WPACK_W], F32R, tag="wpack")
    nc.sync.dma_start(wpack_t[:], wpack)
    w23_t = wpack_t[0:120, W23_OFS:W23_OFS + 1152]
    selA_t = wpack_t[0:96, SELA_OFS:SELA_OFS + 24]
    selB_t = wpack_t[0:96, SELB_OFS:SELB_OFS + 24]
    w5_t = wpack_t[0:64, W5_OFS:W5_OFS + 288]

    # --- persistent planes [128, 2048]: y -> (y%128, (y//128)*512 + x)
    # (tiles created now; DMAs deferred into the strip loop so the first
    # b0 loads aren't stuck behind 4 MiB of plane traffic on SP)
    mos_p = [pp.tile([128, 2048], F32, tag=f"mos{c}", name=f"mos{c}")
             for c in range(4)]

    def load_mos_plane(c):
        nc.scalar.dma_start(
            mos_p[c][:].rearrange("p (t x) -> p t x", t=4),
            mospad[c, 8:520, 1:513].bitcast(F32).rearrange(
                "(t p) x -> p t x", p=128))
    g3 = pp.tile([128, 3 * 2048], F32, tag="g3")
    g0n = g3[:, 0:2048]
    g1n = g3[:, 2048:4096]
    den = g3[:, 4096:6144]
    # chroma c1|c2 interleaved: free = cc*2048 + chunk*512 + x
    ccp = pp.tile([128, 4096], F32R, tag="ccp")
    zt = pp.tile([96, 1028], F32R, tag="zt")
    nc.gpsimd.memset(zt[:].bitcast(F32), 0.0)

    # --- phase 1+2 wavefront over strips
    b0_t, bt = {}, {}  # s -> tile handles; bt: (s, tag)

    def load_b0(s):
        # dual copy at partitions 0:40 and 64:104 for L1 PE row-tiling
        t = b0p.tile([128, 514], F32R, tag="b0", name="b0")
        src = mospad[:, 8 * s + 8:8 * s + 18, :].transpose([1, 0, 2])
        d0 = nc.sync.dma_start(t[0:40, :], src)
        d1 = nc.gpsimd.tensor_copy(t[64:104, :], t[0:40, :])
        add_dep(d1, d0, reason="b0-dup")
        b0_t[s] = (t, [d0, d1])

    def conv_l1(s):
        # two concurrent K=40 row-tiles: f at rows 0:40, w at rows 64:104
        t, deps = b0_t[s]
        psum = ps.tile([96, 1024], F32, tag="cps", name="cps")
        for st in range(2):
            for dx in range(3):
                mm = nc.tensor.matmul(
                    psum[:, st * 512:(st + 1) * 512],
                    wpack_t[64 * st:64 * st + 40,
                            st * 288 + dx * 96:st * 288 + (dx + 1) * 96],
                    t[64 * st:64 * st + 40, dx:dx + 512],
                    start=(dx == 0), stop=(dx == 2))
                for dep in deps:
                    add_dep(mm, dep, reason="rhs-ready")
        return psum

    def conv_pair(tag_in, s, ly_f, ly_w):
        # one layer for both stacks from a fused input tile [120, 1028]
        t, insts = bt[(s, tag_in)]
        psum = ps.tile([96, 1024], F32, tag="cps", name="cps")
        for st, ly in ((0, ly_f), (1, ly_w)):
            for dx in range(3):
                mm = nc.tensor.matmul(
                    psum[:, st * 512:(st + 1) * 512],
                    w23_t[:, ly * 288 + dx * 96:ly * 288 + (dx + 1) * 96],
                    t[0:120, st * 514 + dx:st * 514 + dx + 512],
                    start=(dx == 0), stop=(dx == 2))
                for dep in insts:
                    add_dep(mm, dep, reason="rhs-ready")
        return psum

    def evict_pair(psum, s, tag, k):
        # strip rows m=0..7 hold y = 8s+k+m; rows outside [0,512) must be
        # exactly zero (conv zero-padding) or they leak into the next layer
        t = pools[tag].tile([120, 1028], F32R, tag=tag, name=tag)
        tv = t[0:96, :].rearrange("p (st x) -> p st x", st=2)
        a = nc.scalar.activation(
            tv[:, :, 1:513],
            psum[:].rearrange("p (st x) -> p st x", st=2), RELU)
        z0 = nc.gpsimd.memset(tv[:, :, 0:514:513].bitcast(F32), 0.0)
        add_dep(z0, a, reason="pad-cols")
        insts = [a, z0]
        if s == -1:
            z = nc.gpsimd.memset(t[0:(8 - k) * 12, :].bitcast(F32), 0.0)
            add_dep(z, a, reason="zero-pad-rows")
            add_dep(z, z0, reason="zero-pad-rows")
            insts.append(z)
        if s == 63:
            z = nc.sync.dma_start(t[(8 - k) * 12:96, :], zt[0:k * 12, :])
            add_dep(z, a, reason="zero-pad-rows")
            add_dep(z, z0, reason="zero-pad-rows")
            insts.append(z)
        bt[(s, tag)] = (t, insts)

    def halo(s, tag):
        # bt[(s,tag)][96:120] <- bt[(s+1,tag)][0:24]  (rows y+8, y+9)
        # partition-shifted engine copy (bases 96 and 0 are 32-aligned)
        dst, insts = bt[(s, tag)]
        eng = nc.vector.tensor_copy
        if (s + 1, tag) in bt:
            src_t, src_insts = bt[(s + 1, tag)]
            d = eng(dst[96:120, :], src_t[0:24, :])
            for i_ in src_insts:
                add_dep(d, i_, reason="halo-src-ready")
            for i_ in insts:
                add_dep(d, i_, reason="halo-after-evict")
        else:
            # zeros into rows 96:120 are partition-disjoint from the
            # eviction's rows 0:96 -- no ordering needed beyond slot WAR
            nc.gpsimd.memset(dst[96:120, :].bitcast(F32), 0.0)

    def phase2(s, pair):
        it = ph2.tile([96, 512], F32R, tag="i")
        et = ph2.tile([96, 512], F32R, tag="e")
        nc.scalar.activation(it[:], pair[:, 0:512], RELU)
        nc.scalar.activation(et[:], pair[:, 512:1024], EXP)
        # max(exp(x), 1) == exp(relu(x)) -- this IS the w-stack's last relu
        nc.vector.tensor_scalar_max(et[:], et[:], 1.0)
        nc.vector.tensor_mul(it[:], it[:], et[:])  # i*E in place
        p2 = ps23.tile([24, 512], F32, tag="p2x", name="p2", bufs=1)[0:24, :]
        nc.tensor.matmul(p2[:], selA_t, it[:], start=True, stop=False)
        nc.tensor.matmul(p2[:], selB_t, et[:], start=False, stop=True)
        s2 = stgp.tile([24, 512], F32, tag="stg", name="stg")
        nc.vector.tensor_copy(s2[:], p2[:])
        ys = 8 * s + 3
        ya, yb = max(ys, 0), min(ys + 8, HW)
        while ya < yb:
            run = min(yb - ya, 128 - (ya % 128))
            p0 = ya % 128
            dst = g3[p0:p0 + run, :].rearrange(
                "p (v c x) -> p v c x", v=3, c=4)[:, :, ya // 128, :]
            sv = s2[(ya - ys) * 3:(ya - ys + run) * 3, :]
            (nc.scalar if s >= 61 else nc.sync).dma_start(dst, sv)
            ya += run

    # --- phases 2.5/3/4, interleaved into the wavefront
    asm_specs = [  # (ch, py, px, qp index or None, plane addend or None)
        (0, 0, 0, 0, mos_p[0]), (0, 0, 1, None, mos_p[1]),
        (0, 1, 0, 4, g1n), (0, 1, 1, 3, mos_p[3]),
        (1, 0, 0, None, mos_p[0]), (1, 0, 1, None, g0n),
        (1, 1, 0, None, g1n), (1, 1, 1, None, mos_p[3]),
        (2, 0, 0, 2, mos_p[0]), (2, 0, 1, 5, g0n),
        (2, 1, 0, None, mos_p[2]), (2, 1, 1, 1, mos_p[3]),
    ]
    qp6_q = {}

    # persistent pre-zeroed b3 rhs buffers: rows 18..31 / 50..63 and pad
    # cols 0,513 stay zero forever; gathers only write rows 0..17/32..49,
    # cols 1:513.  sq==0 leaves rows 0,32 pristine (buf first use); sq==31
    # gets its own buf 4 so rows 17,49 stay pristine.
    b3_bufs = []
    for i in range(5):
        t = b3p.tile([64, 514], F32R, tag=f"b3_{i}", name=f"b3_{i}")
        nc.gpsimd.memset(t[:].bitcast(F32), 0.0)
        b3_bufs.append(t)

    def phase25(t, q):
        # green + chroma for y rows 128t+32q .. 128t+32q+31 (partition quarter
        # q of free chunk t); complete right after strip 16t+4q+3's scatter
        f0 = t * 512
        pr = slice(32 * q, 32 * q + 32)

        def g3v(v):
            return g3[pr, v * 2048 + f0:v * 2048 + f0 + 512]

        nc.vector.reciprocal(g3v(2), g3v(2))
        nc.vector.tensor_mul(g3v(0), g3v(0), g3v(2))  # g0
        nc.vector.tensor_mul(g3v(1), g3v(1), g3v(2))  # g1
        nc.vector.tensor_sub(ccp[pr, f0:f0 + 512],
                             mos_p[1][pr, f0:f0 + 512], g3v(0))
        nc.vector.tensor_sub(ccp[pr, 2048 + f0:2048 + f0 + 512],
                             mos_p[2][pr, f0:f0 + 512], g3v(1))

    def phase3_strip(sq):
        qt = sq // 8
        if qt not in qp6_q:
            qp6_q[qt] = qpp.tile([128, 6 * 512], F32, tag="qp6",
                                 name=f"qp6_{qt}")
        qp6 = qp6_q[qt]
        b3 = b3_bufs[4 if sq == 31 else sq % 4]
        y0 = 16 * sq - 1
        ya, yb = max(y0, 0), min(y0 + 18, HW)
        wrs = []
        while ya < yb:
            run = min(yb - ya, 128 - (ya % 128))
            p0 = ya % 128
            tlc = ya // 128
            for cc in range(2):
                eng = nc.sync if cc == 0 else nc.scalar
                d = eng.dma_start(
                    b3[cc * 32 + ya - y0:cc * 32 + ya - y0 + run, 1:513],
                    ccp[p0:p0 + run,
                        cc * 2048 + tlc * 512:cc * 2048 + tlc * 512 + 512])
                wrs.append(d)
            ya += run
        p3 = ps23.tile([96, 512], F32, tag="p23", name="p3", bufs=1)
        mm3 = [nc.tensor.matmul(p3[:], w5_t[:, dx * 96:(dx + 1) * 96],
                                b3[0:64, dx:dx + 512],
                                start=(dx == 0), stop=(dx == 2))
               for dx in range(3)]
        for mm in mm3:
            for wr in wrs:
                add_dep(mm, wr, reason="b3-ready")
        s3 = ph2.tile([96, 512], F32, tag="s3")
        nc.vector.tensor_copy(s3[:], p3[:])
        yq = 16 * sq
        qeng = (nc.gpsimd, nc.sync, nc.scalar)[(sq + 1) % 3] if sq >= 29 \
            else nc.gpsimd
        qeng.dma_start(
            qp6[yq % 128:yq % 128 + 16, :].rearrange("p (o x) -> p o x", o=6),
            s3[:])

    def assemble_quarter(t):
        qp6 = qp6_q.pop(t)
        for ch in range(3):
            a = asmp.tile([128, 2048], F32, tag="asm", name="asm")
            prev = None
            for (c_, py, px, qo, addend) in asm_specs:
                if c_ != ch:
                    continue
                view = a[:].rearrange("p (py x two) -> p py two x",
                                      py=2, two=2)[:, py, px, :]
                if qo is None:
                    src = addend[:, t * 512:(t + 1) * 512]
                    nc.gpsimd.tensor_copy(view, src)
                else:
                    nc.vector.tensor_add(
                        view, qp6[:, qo * 512:qo * 512 + 512],
                        addend[:, t * 512:(t + 1) * 512])
            # out[ch] rows 256t..256t+255 are exactly a[:] flattened
            dst = out[ch][256 * t:256 * (t + 1), :].rearrange(
                "(p f) x -> p (f x)", f=2)
            nc.sync.dma_start(dst, a[:])

    # interleave: after phase2(t3) finishes the last strip of plane-chunk t
    # (t3 == 16t+15), run that chunk's green/chroma and enqueue the phase-3
    # strips it unlocks; they're then drip-fed one per strip iteration so PE
    # always has conv matmuls between the gather-dependent phase-3 matmuls.
    pending = []
    PDELAY = int(os.environ.get("K_PDELAY", "13"))
    PCLAMP = int(os.environ.get("K_PCLAMP", "66"))

    next_sq = [0]
    sched = [0]

    def emit_quarter_chunk(t, q, ready_i):
        phase25(t, q)
        # ccp rows <= 128t+32q+31 now ready; phase-3 strip sq needs rows
        # <= 16sq+16, so strips up to 8t+2q+1 are unlocked
        hi = min(8 * t + 2 * q + 1, 31)
        for sq in range(next_sq[0], hi + 1):
            r = max(ready_i, sched[0] + 1)
            sched[0] = r
            pending.append((r, "sq", sq))
            if sq % 8 == 7:
                pending.append((r, "asm", sq // 8))
        next_sq[0] = max(next_sq[0], hi + 1)

    def run_pending(now_i, n):
        while pending and n > 0 and pending[0][0] <= now_i:
            _, kind, arg = pending.pop(0)
            if kind == "sq":
                phase3_strip(arg)
            else:
                assemble_quarter(arg)
            n -= 1

    load_b0(-1)
    for i in range(NSTRIP + 4):
        s = i - 1  # L1 strip index
        if s <= 63:
            if s + 1 <= 63:
                load_b0(s + 1)  # prefetch next strip's input
            evict_pair(conv_l1(s), s, "b1", 1)
            if s - 1 >= -1:
                b0_t.pop(s - 1, None)
        if i == 3:
            for c in range(4):
                load_mos_plane(c)
        t2 = s - 2  # L2 strip index (skewed: halo source already evicted)
        if -1 <= t2 <= 63:
            halo(t2, "b1")
            evict_pair(conv_pair("b1", t2, 0, 1), t2, "b2", 2)
        t3 = s - 4  # L3 strip index (skewed)
        if -1 <= t3 <= 63:
            halo(t3, "b2")
            phase2(t3, conv_pair("b2", t3, 2, 3))
            bt.pop((t3, "b1"), None)
            if t3 - 1 >= -1:
                bt.pop((t3 - 1, "b2"), None)
            if t3 >= 3 and t3 % 4 == 3:
                # items become runnable PDELAY iterations after their emit,
                # but clamp near the loop end so late-chunk items interleave
                # with the L1-drained final iterations instead of bunching
                # into the serial post-loop drain
                emit_quarter_chunk(t3 // 16, (t3 % 16) // 4,
                                   max(min(i + PDELAY, PCLAMP), i + 2))
        run_pending(i, 1)
    run_pending(10 ** 9, len(pending))


_CACHE = {}


def _get_compiled():
    if "nc" in _CACHE:
        return _CACHE["nc"]
    nc = bacc.Bacc("TRN2", target_bir_lowering=False, debug=False,
                   enable_asserts=False)
    ins = {
        "mospad": nc.dram_tensor("mospad", [4, 522, 514], F32R,
                                 kind="ExternalInput").ap(),
        "wpack": nc.dram_tensor("wpack", [128, WPACK_W], F32R,
                                kind="ExternalInput").ap(),
    }
    outs = {"out": nc.dram_tensor("out", [3, 1024, 1024], F32,
                                  kind="ExternalOutput").ap()}
    from contextlib import ExitStack
    with tile.TileContext(nc) as tc, ExitStack() as ctx:
        build_kernel(tc, outs, ins, ctx)
    nc.compile()
    _CACHE["nc"] = nc
    return nc


def kernel(**inputs):
    nc = _get_compiled()
    mospad, shared = _host_prep(inputs)
    in_maps = []
    for b in range(8):
        m = {"mospad": np.ascontiguousarray(mospad[b])}
        m.update(shared)
        in_maps.append(m)
    res = run_bass_kernel_spmd(nc, in_maps, core_ids=list(range(8)))
    return np.stack([res.results[b]["out"] for b in range(8)])
